# revision 30
# baseline (speedup 1.0000x reference)
"""8-core Trainium2 Bass kernel for nn_MetabolicGNN (GCN x2 + GAT + MLP).

Strategy: nodes permuted into 392 degree-balanced tiles of 128 (49/core);
per layer a node table (fp16) is built as core shards and AllGathered in
two segments A (tiles 0-23 of each core) / B (tiles 24-48) so each segment
overlaps surrounding compute and the int16 dma_gather indices stay in
range. Edges grouped by dst tile and src segment; per tile src rows are
pulled with dma_gather (<=1024 idx/instr, 4 SWDGE queues). Aggregation per
128-edge chunk is a PE matmul with a host-precomputed fp8 one-hot
[edge x dst] as stationary operand. GCN norm folded (dinv[src] row scale,
dinv[dst] output scale). GAT: rows carry [4x128 feats | a_s f32x4 | pad];
per-edge a_d comes from an fp8 [dst x edge] one-hot matmul; attention
weights ex=exp(leaky(a_s+a_d)) are applied by scaling the gathered rows in
ONE tensor_tensor op per tile (broadcast over head feature blocks), with
ex appended so the same one-hot matmul accumulates numerators and softmax
denominators together.
"""
import sys

sys.path.insert(0, "/opt/trn_rl_repo")

import numpy as np

N = 50000
E = 800000
IN_DIM, HID, OUT_DIM, HEADS = 256, 128, 64, 4
NCORES = 8
P = 128
TPC = 49                    # tiles per core
TA = 24                     # A-segment tiles per core (B = TPC-TA)
TB = TPC - TA
NT = NCORES * TPC           # 392 tiles
N_PAD = NT * P              # 50176
NPC = TPC * P               # 6272 nodes per core
NPA = TA * P                # 3072
NPB = TB * P                # 3200
SPLA = NCORES * NPA         # 24576 rows in table A (int16-safe)
SPLB = NCORES * NPB         # 25600 rows in table B
GD = HEADS * HID            # 512
GROW = 640                  # GAT row: 512 feats + 8 (a_s f32x4) + pad, %256B
AS_OFF = 512
MAXI = 1024                 # dma_gather per-instruction index cap (ucode)
GBUFS = 6                   # max gather-pool bufs (maiden-tile count)
RING = 6                    # gather count-register ring depth (tiles)


def _side_groups(c):
    """Split c chunks into <=8-chunk gather groups, near-equal sizes."""
    assert c <= 16
    if c <= 8:
        return [(0, c)]
    h = (c + 1) // 2
    return [(0, h), (h, c - h)]


def _wrap16(stream):
    """[L] int array (L%16==0) -> [128, L//16] int16, 16-wrapped, x8 replicated."""
    w = stream.reshape(-1, 16).T
    return np.tile(w, (8, 1)).astype(np.int16)


def _preprocess(edge_index):
    import heapq
    src = edge_index[0].astype(np.int64)
    dst = edge_index[1].astype(np.int64)
    loop = np.arange(N, dtype=np.int64)
    srcA = np.concatenate([src, loop])
    dstA = np.concatenate([dst, loop])
    deg = np.bincount(dstA, minlength=N).astype(np.int64)
    dinv = (1.0 / np.sqrt(deg)).astype(np.float32)

    # ---- pass 1: degree-balanced tiles (defines node halves) ----
    order = np.argsort(-deg, kind="stable")
    fill = np.zeros(NT, np.int64)
    load = np.zeros(NT, np.int64)
    t1 = np.full(N, -1, np.int64)
    heap = [(0, i) for i in range(NT)]
    heapq.heapify(heap)
    for n in order:
        while True:
            ld, i = heapq.heappop(heap)
            if fill[i] < P:
                break
        t1[n] = i
        fill[i] += 1
        load[i] += deg[n]
        if fill[i] < P:
            heapq.heappush(heap, (load[i], i))
    half = ((t1 % TPC) >= TA).astype(np.int8)   # A: tl<TA, B: tl>=TA

    # per-node (A,B) in-edge profile under pass-1 halves
    e_half = half[srcA]
    dl = np.bincount(dstA, weights=(e_half == 0).astype(np.float64),
                     minlength=N).astype(np.int64)
    dh = np.bincount(dstA, weights=(e_half == 1).astype(np.float64),
                     minlength=N).astype(np.int64)

    # ---- pass 2: per-half reassignment balancing max(A,B) ----
    node_tile = np.full(N_PAD, -1, np.int64)
    node_slot = np.full(N_PAD, -1, np.int64)
    for hf in (0, 1):
        if hf == 0:
            tiles = [c * TPC + tl for c in range(NCORES) for tl in range(TA)]
        else:
            tiles = [c * TPC + tl for c in range(NCORES) for tl in range(TA, TPC)]
        nodes = np.where(half == hf)[0]
        key = dl[nodes] + dh[nodes]
        nodes = nodes[np.argsort(-key, kind="stable")]
        nt = len(tiles)
        fill2 = np.zeros(nt, np.int64)
        llo = np.zeros(nt, np.int64)
        lhi = np.zeros(nt, np.int64)
        heap = [(0, i) for i in range(nt)]
        heapq.heapify(heap)
        for n in nodes:
            while True:
                _, i = heapq.heappop(heap)
                if fill2[i] < P:
                    break
            node_tile[n] = tiles[i]
            node_slot[n] = fill2[i]
            fill2[i] += 1
            llo[i] += dl[n]
            lhi[i] += dh[n]
            if fill2[i] < P:
                heapq.heappush(heap, (max(llo[i], lhi[i]), i))

    # pad node ids fill all remaining slots
    used = np.zeros((NT, P), bool)
    for n in range(N):
        used[node_tile[n], node_slot[n]] = True
    free_all = [(t, s) for t in range(NT) for s in range(P) if not used[t, s]]
    assert len(free_all) == N_PAD - N
    for pn, (t, s) in zip(range(N, N_PAD), free_all):
        node_tile[pn] = t
        node_slot[pn] = s

    perm = node_tile * P + node_slot            # old id -> new id (core-major)

    # side-relative row index for each new node id
    n_core = node_tile // TPC
    n_tl = node_tile % TPC
    n_isB = (n_tl >= TA)
    side_row = np.where(
        n_isB,
        n_core * NPB + (n_tl - TA) * P + node_slot,
        n_core * NPA + n_tl * P + node_slot)

    # ---- edge lists per tile, A-side first ----
    e_tile = node_tile[dstA]
    e_slot = node_slot[dstA]
    e_isB = n_isB[srcA]
    e_srow = side_row[srcA]
    eo = np.lexsort((e_isB, e_tile))   # by tile, A first
    e_tile, e_slot, e_isB, e_srow = (
        e_tile[eo], e_slot[eo], e_isB[eo], e_srow[eo])
    starts = np.searchsorted(e_tile, np.arange(NT))
    ends = np.searchsorted(e_tile, np.arange(NT) + 1)

    nlow = np.zeros(NT, np.int64)
    nhigh = np.zeros(NT, np.int64)
    for t in range(NT):
        s, e = starts[t], ends[t]
        nlow[t] = int((~e_isB[s:e]).sum())
        nhigh[t] = (e - s) - nlow[t]
    cA = int(-(-nlow.max() // P))
    cB = int(-(-nhigh.max() // P))
    CPT = cA + cB

    epc = CPT * P
    dst_slot = np.full((NCORES, TPC * epc), -1.0, dtype=np.float32)
    sidx = np.full((NCORES, TPC * epc), -1, dtype=np.int64)  # in-side row idx
    nl_ct = np.zeros((NCORES, TPC), np.int64)
    nh_ct = np.zeros((NCORES, TPC), np.int64)
    for t in range(NT):
        c, tl = divmod(t, TPC)
        s, e = starts[t], ends[t]
        nl = int(nlow[t])
        nh = (e - s) - nl
        nl_ct[c, tl] = nl
        nh_ct[c, tl] = nh
        base = tl * epc
        sidx[c, base:base + nl] = e_srow[s:s + nl]
        sidx[c, base + cA * P:base + cA * P + nh] = e_srow[s + nl:e]
        dst_slot[c, base:base + nl] = e_slot[s:s + nl]
        dst_slot[c, base + cA * P:base + cA * P + nh] = e_slot[s + nl:e]

    # ---- gather groups, per-group valid counts, -1 pads with 0 sentinels ----
    groups = [(0, ch0, nch) for ch0, nch in _side_groups(cA)] + \
             [(1, cA + ch0, nch) for ch0, nch in _side_groups(cB)]
    # Full-capacity gathers everywhere: pads use sentinel row 0, so gather
    # buffers are completely overwritten every tile (stale-data safe, and
    # count registers are loop-invariant).
    counts = np.zeros((NCORES, TPC * 8), np.int32)
    for c in range(NCORES):
        for tl in range(TPC):
            base = tl * epc
            for gi, (hf, ch0, nch) in enumerate(groups):
                g0 = base + ch0 * P
                cap = nch * P
                nv = int(nl_ct[c, tl]) - ch0 * P if hf == 0 else \
                    int(nh_ct[c, tl]) - (ch0 - cA) * P
                nv = max(0, min(cap, nv))
                sidx[c, g0 + nv:g0 + cap] = 0
                counts[c, tl * 8 + gi] = cap

    # ---- wrapped int16 streams ----
    scols = epc // 16                   # idx cols per tile
    sidx16 = np.zeros((NCORES, 128, TPC * scols), np.int16)
    for c in range(NCORES):
        for tl in range(TPC):
            base = tl * epc
            col0 = tl * scols
            sidx16[c][:, col0:col0 + scols] = _wrap16(sidx[c, base:base + epc])

    dinv_new = np.ones(N_PAD, dtype=np.float32)
    dinv_new[perm[:N]] = dinv

    # ---- host-precomputed fp8 one-hot blocks: oh [e,d] and ohT [d,e] ----
    import ml_dtypes
    f8 = ml_dtypes.float8_e4m3
    ohd = np.empty((NCORES, 128, TPC * CPT * P), f8)
    ohdT = np.empty((NCORES, 128, TPC * CPT * P), f8)
    rng = np.arange(P, dtype=np.float32)
    for c in range(NCORES):
        ds = dst_slot[c].reshape(TPC, CPT, P)          # [t, j, e]
        oh = (ds[:, :, :, None] == rng)                 # [t, j, e, d]
        ohd[c] = np.ascontiguousarray(
            oh.transpose(2, 0, 1, 3).reshape(P, -1)).astype(f8)
        ohdT[c] = np.ascontiguousarray(
            oh.transpose(3, 0, 1, 2).reshape(P, -1)).astype(f8)

    plan = {"cA": cA, "cB": cB, "CPT": CPT, "groups": groups}
    return sidx16, dinv_new, perm, plan, counts, ohd, ohdT


def _build_nc(plan):
    import concourse.bass as bass
    import concourse.bacc as bacc
    import concourse.tile as tile
    from concourse import mybir

    f32 = mybir.dt.float32
    fp16 = mybir.dt.float16
    f8 = mybir.dt.float8e4
    i16 = mybir.dt.int16
    i32 = mybir.dt.int32
    AF = mybir.ActivationFunctionType
    OP = mybir.AluOpType

    cA, cB, CPT = plan["cA"], plan["cB"], plan["CPT"]
    groups = plan["groups"]
    SCOLS = CPT * P // 16

    nc = bacc.Bacc(trn_type="TRN2", target_bir_lowering=False,
                   num_devices=NCORES, dynamic_dma_scratch_size=16384,
                   num_swdge_queues=4)

    # ---- I/O ----
    xT_d = nc.dram_tensor("xT_c", [P, 2 * NPC], fp16, kind="ExternalInput")
    sidx_d = nc.dram_tensor("sidx16", [P, TPC * SCOLS], i16, kind="ExternalInput")
    cnt_d = nc.dram_tensor("counts", [P, TPC * 8], i32, kind="ExternalInput")
    ohd_d = nc.dram_tensor("ohd", [P, TPC * CPT * P], f8, kind="ExternalInput")
    ohdT_d = nc.dram_tensor("ohdT", [P, TPC * CPT * P], f8, kind="ExternalInput")
    dinv_d = nc.dram_tensor("dinv_t", [P, TPC], f32, kind="ExternalInput")
    ident_d = nc.dram_tensor("ident16", [P, P], fp16, kind="ExternalInput")
    win_d = nc.dram_tensor("WinT", [P, 2 * HID], fp16, kind="ExternalInput")
    bin_d = nc.dram_tensor("bin_pp", [P, 1], f32, kind="ExternalInput")
    wg1_d = nc.dram_tensor("Wg1", [P, HID], fp16, kind="ExternalInput")
    wg2_d = nc.dram_tensor("Wg2", [P, HID], fp16, kind="ExternalInput")
    bg1_d = nc.dram_tensor("bg1_bc", [P, HID], f32, kind="ExternalInput")
    bg2_d = nc.dram_tensor("bg2_bc", [P, HID], f32, kind="ExternalInput")
    g1g_d = nc.dram_tensor("g1g_bc", [P, HID], f32, kind="ExternalInput")
    g1b_d = nc.dram_tensor("g1b_bc", [P, HID], f32, kind="ExternalInput")
    g2g_d = nc.dram_tensor("g2g_bc", [P, HID], f32, kind="ExternalInput")
    g2b_d = nc.dram_tensor("g2b_bc", [P, HID], f32, kind="ExternalInput")
    wgat_d = nc.dram_tensor("Wgat", [P, GD], fp16, kind="ExternalInput")
    vsvd_d = nc.dram_tensor("VsVd", [P, 2 * HEADS], fp16, kind="ExternalInput")
    watt_d = nc.dram_tensor("WattT", [P, GD // P, HID], fp16, kind="ExternalInput")
    batt_d = nc.dram_tensor("batt_pp", [P, 1], f32, kind="ExternalInput")
    wout_d = nc.dram_tensor("Wout", [P, OUT_DIM], fp16, kind="ExternalInput")
    bout_d = nc.dram_tensor("bout_bc", [P, OUT_DIM], f32, kind="ExternalInput")
    eps_d = nc.dram_tensor("eps_pp", [P, 1], f32, kind="ExternalInput")
    out_c = nc.dram_tensor("out_c", [NPC, OUT_DIM], f32, kind="ExternalOutput")

    # ---- internal DRAM ----
    def seg_pair(name, w):
        a_in = nc.dram_tensor(name + "Ai", [NPA, w], fp16, kind="Internal")
        b_in = nc.dram_tensor(name + "Bi", [NPB, w], fp16, kind="Internal")
        a_tb = nc.dram_tensor(name + "A", [SPLA, w], fp16, kind="Internal",
                              addr_space="Shared")
        b_tb = nc.dram_tensor(name + "B", [SPLB, w], fp16, kind="Internal",
                              addr_space="Shared")
        return a_in, b_in, a_tb, b_tb

    t1Ai, t1Bi, t1A, t1B = seg_pair("tb1", HID)
    t2Ai, t2Bi, t2A, t2B = seg_pair("tb2", HID)
    t3Ai, t3Bi, t3A, t3B = seg_pair("tb3", GROW)

    rg = [list(range(NCORES))]

    with tile.TileContext(nc) as tc:
        with (
            tc.tile_pool(name="const", bufs=1) as cpool,
            tc.tile_pool(name="big", bufs=1) as bigpool,
            tc.tile_pool(name="work", bufs=3) as wpool,
            tc.tile_pool(name="small", bufs=4) as spool,
            tc.tile_pool(name="ohs", bufs=3) as ohpool,
            tc.tile_pool(name="ps", bufs=2, space="PSUM") as pspool,
            tc.tile_pool(name="pst", bufs=2, space="PSUM") as tppool,
        ):
            # ---------- constants ----------
            def cload(dram, shape, dtype=f32):
                t = cpool.tile(shape, dtype, tag="c_" + dram.name)
                nc.sync.dma_start(out=t[:], in_=dram[:])
                return t

            ident_t = cload(ident_d, [P, P], fp16)
            win_t = cpool.tile([P, 2, HID], fp16, tag="c_WinT")
            nc.sync.dma_start(out=win_t[:],
                              in_=win_d[:].rearrange("p (h c) -> p h c", h=2))
            bin_t = cload(bin_d, [P, 1])
            wg1_t = cload(wg1_d, [P, HID], fp16)
            wg2_t = cload(wg2_d, [P, HID], fp16)
            bg1_t = cload(bg1_d, [P, HID])
            bg2_t = cload(bg2_d, [P, HID])
            g1g_t = cload(g1g_d, [P, HID])
            g1b_t = cload(g1b_d, [P, HID])
            g2g_t = cload(g2g_d, [P, HID])
            g2b_t = cload(g2b_d, [P, HID])
            wgat_t = cload(wgat_d, [P, GD], fp16)
            vsvd_t = cload(vsvd_d, [P, 2 * HEADS], fp16)
            watt_t = cload(watt_d, [P, GD // P, HID], fp16)
            batt_t = cload(batt_d, [P, 1])
            wout_t = cload(wout_d, [P, OUT_DIM], fp16)
            bout_t = cload(bout_d, [P, OUT_DIM])
            eps_t = cload(eps_d, [P, 1])
            dinv_t = cload(dinv_d, [P, TPC])
            sidx = cload(sidx_d, [P, TPC * SCOLS], i16)
            cnt = cload(cnt_d, [P, TPC * 8], i32)
            gregs = [nc.alloc_register(mybir.EngineType.Pool, f"gcnt{i}")
                     for i in range(len(groups))]
            a_d_all = cpool.tile([P, TPC * HEADS], fp16, tag="c_adall")

            h0T = bigpool.tile([P, NPC], fp16, tag="h0T")
            x1T = bigpool.tile([P, NPC], fp16, tag="x1T")

            # ---------- P1: h0T = relu(Win.T @ x.T + bin), feature-major ----
            for ch in range(13):
                n0 = ch * 512
                nn = min(512, NPC - n0)
                xt = wpool.tile([P, 2, 512], fp16, tag="xload")
                nc.sync.dma_start(
                    out=xt[:, :, :nn],
                    in_=xT_d[:].rearrange("p (h n) -> p h n", h=2)[:, :, n0:n0 + nn])
                hp = pspool.tile([P, 512], f32, tag="mm")
                for h in range(2):
                    nc.tensor.matmul(out=hp[:, :nn], lhsT=win_t[:, h, :],
                                     rhs=xt[:, h, :nn],
                                     start=(h == 0), stop=(h == 1))
                nc.scalar.activation(out=h0T[:, n0:n0 + nn], in_=hp[:, :nn],
                                     func=AF.Relu, bias=bin_t[:], scale=1.0)

            # ---------- helper: xw table build + segmented AG ----------
            def build_table(srcT, w_t, ains, tbs):
                (aA, aB), (tA, tB_) = ains, tbs
                for t in range(TPC):
                    ps = pspool.tile([P, HID], f32, tag="mm")
                    nc.tensor.matmul(out=ps[:], lhsT=srcT[:, t * P:(t + 1) * P],
                                     rhs=w_t[:], start=True, stop=True)
                    sb = wpool.tile([P, HID], fp16, tag="xwsb")
                    nc.vector.tensor_scalar(out=sb[:], in0=ps[:],
                                            scalar1=dinv_t[:, t:t + 1],
                                            scalar2=None, op0=OP.mult)
                    if t < TA:
                        nc.sync.dma_start(out=aA[t * P:(t + 1) * P, :], in_=sb[:])
                    else:
                        tb0 = t - TA
                        nc.sync.dma_start(out=aB[tb0 * P:(tb0 + 1) * P, :], in_=sb[:])
                    if t == TA - 1:
                        nc.gpsimd.collective_compute(
                            "AllGather", OP.bypass, ins=[aA[:]], outs=[tA[:]],
                            replica_groups=rg)
                nc.gpsimd.collective_compute(
                    "AllGather", OP.bypass, ins=[aB[:]], outs=[tB_[:]],
                    replica_groups=rg)

            # ---------- helper: gathers for one tile from a table pair ----
            NG = len(groups)
            # capacity counts are loop-invariant: load the registers once
            nc.gpsimd.reg_load(gregs[:NG], cnt[0:1, 0:NG])

            def tile_gathers(gt, tA, tB_, t, width, qbase):
                for gi, (hf, ch0, nch) in enumerate(groups):
                    ni = nch * P
                    col0 = t * SCOLS + ch0 * 8
                    view = tA[:] if hf == 0 else tB_[:]
                    nc.gpsimd.dma_gather(
                        gt[:, ch0:ch0 + nch, :], view,
                        sidx[:, col0:col0 + nch * 8], ni, gregs[gi], width,
                        queue_num=(qbase + gi) % 4)

            # ---------- helper: layernorm(+relu) on node-major tile ----------
            def ln_relu(dst, src, gam, bet):
                st = spool.tile([P, 6], f32, tag="lnst")
                nc.vector.bn_stats(out=st[:], in_=src[:])
                mv = spool.tile([P, 2], f32, tag="lnmv")
                nc.vector.bn_aggr(out=mv[:], in_=st[:])
                rstd = spool.tile([P, 1], f32, tag="lnrs")
                nc.scalar.activation(out=rstd[:], in_=mv[:, 1:2], func=AF.Sqrt,
                                     bias=eps_t[:], scale=1.0)
                nc.vector.reciprocal(out=rstd[:], in_=rstd[:])
                nmb = spool.tile([P, 1], f32, tag="lnnb")
                nc.vector.tensor_scalar(out=nmb[:], in0=mv[:, 0:1],
                                        scalar1=rstd[:, 0:1], scalar2=-1.0,
                                        op0=OP.mult, op1=OP.mult)
                nrm = wpool.tile([P, HID], f32, tag="lnnrm")
                nc.scalar.activation(out=nrm[:], in_=src[:], func=AF.Identity,
                                     bias=nmb[:], scale=rstd[:])
                nc.vector.tensor_mul(out=nrm[:], in0=nrm[:], in1=gam[:])
                nc.vector.tensor_add(out=nrm[:], in0=nrm[:], in1=bet[:])
                nc.scalar.activation(out=dst[:], in_=nrm[:], func=AF.Relu)

            # ---------- helper: GCN aggregation pass (3-stage SW pipeline) ----
            def gcn_pass(gpool, upool, tbs, bg_t, gam, bet, outT, resT, post=None):
                tA, tB_ = tbs
                gts, Us = {}, {}

                def s0(t):
                    gt = gpool.tile([P, CPT, HID], fp16, tag="gcng")
                    tile_gathers(gt, tA, tB_, t, HID, qbase=t)
                    oh = ohpool.tile([P, CPT * P], f8, tag="ohd")
                    nc.sync.dma_start(out=oh[:],
                                      in_=ohd_d[:, t * CPT * P:(t + 1) * CPT * P])
                    gts[t] = (gt, oh)

                def s1(t):
                    gt, oh = gts[t]
                    U = upool.tile([P, HID], f32, tag="U1")
                    for j in range(CPT):
                        nc.tensor.matmul(out=U[:], lhsT=oh[:, j * P:(j + 1) * P],
                                         rhs=gt[:, j, :],
                                         start=(j == 0), stop=(j == CPT - 1))
                    Us[t] = U

                def s2(t):
                    U = Us.pop(t)
                    del gts[t]
                    pre = wpool.tile([P, HID], f32, tag="gcnpre")
                    nc.vector.scalar_tensor_tensor(
                        out=pre[:], in0=U[:], scalar=dinv_t[:, t:t + 1], in1=bg_t[:],
                        op0=OP.mult, op1=OP.add)
                    nm = wpool.tile([P, HID], fp16, tag="gcnnm")
                    ln_relu(nm, pre, gam, bet)
                    tp = tppool.tile([P, P], fp16, tag="tp")
                    nc.tensor.transpose(out=tp[:], in_=nm[:], identity=ident_t[:])
                    if resT is None:
                        nc.vector.tensor_copy(out=outT[:, t * P:(t + 1) * P], in_=tp[:])
                    else:
                        nc.vector.tensor_add(out=outT[:, t * P:(t + 1) * P],
                                             in0=resT[:, t * P:(t + 1) * P], in1=tp[:])
                    if post is not None:
                        post(t, outT)

                for i in range(TPC + 3):
                    if 3 <= i:
                        s2(i - 3)
                    if 2 <= i < TPC + 2:
                        s1(i - 2)
                    if i < TPC:
                        s0(i)

            # ---------- GCN layer 1 ----------
            build_table(h0T, wg1_t, (t1Ai, t1Bi), (t1A, t1B))
            with (
                tc.tile_pool(name="g1", bufs=GBUFS) as gpool1,
                tc.tile_pool(name="psU1", bufs=4, space="PSUM") as upool1,
            ):
                gcn_pass(gpool1, upool1, (t1A, t1B), bg1_t, g1g_t, g1b_t,
                         x1T, None)

            # ---------- GCN layer 2 (residual) + fused GAT table build ------
            build_table(x1T, wg2_t, (t2Ai, t2Bi), (t2A, t2B))
            x2T = h0T  # reuse slot

            def gat_stage(t, outT):
                ps = pspool.tile([P, GD], f32, tag="mm")
                nc.tensor.matmul(out=ps[:], lhsT=outT[:, t * P:(t + 1) * P],
                                 rhs=wgat_t[:], start=True, stop=True)
                ps8 = tppool.tile([P, 2 * HEADS], f32, tag="tp")
                nc.tensor.matmul(out=ps8[:], lhsT=outT[:, t * P:(t + 1) * P],
                                 rhs=vsvd_t[:], start=True, stop=True)
                stg = wpool.tile([P, GROW], fp16, tag="stg")
                nc.vector.tensor_copy(out=stg[:, 0:GD], in_=ps[:])
                stgf = stg[:, AS_OFF:AS_OFF + 8].bitcast(f32)
                nc.vector.tensor_copy(out=stgf, in_=ps8[:, 0:HEADS])
                nc.vector.tensor_copy(out=a_d_all[:, t * HEADS:(t + 1) * HEADS],
                                      in_=ps8[:, HEADS:2 * HEADS])
                if t < TA:
                    nc.sync.dma_start(out=t3Ai[t * P:(t + 1) * P, :], in_=stg[:])
                else:
                    tb0 = t - TA
                    nc.sync.dma_start(out=t3Bi[tb0 * P:(tb0 + 1) * P, :], in_=stg[:])
                if t == TA - 1:
                    nc.gpsimd.collective_compute(
                        "AllGather", OP.bypass, ins=[t3Ai[:]], outs=[t3A[:]],
                        replica_groups=rg)
                if t == TPC - 1:
                    nc.gpsimd.collective_compute(
                        "AllGather", OP.bypass, ins=[t3Bi[:]], outs=[t3B[:]],
                        replica_groups=rg)

            with (
                tc.tile_pool(name="g2", bufs=GBUFS) as gpool2,
                tc.tile_pool(name="psU2", bufs=4, space="PSUM") as upool2,
            ):
                gcn_pass(gpool2, upool2, (t2A, t2B), bg2_t, g2g_t, g2b_t,
                         x2T, x1T, post=gat_stage)

            # ---------- P7: GAT aggregation + att_out + output proj ----------
            # 4-stage SW pipeline: s0 gathers/streams, s1 attention logits,
            # s2 row scaling + aggregation matmuls, s3 normalize + output.
            with (
                tc.tile_pool(name="g3", bufs=5) as gpool3,
                tc.tile_pool(name="psg", bufs=2, space="PSUM") as psg,
            ):
                gts, exs, ohs, Uabs = {}, {}, {}, {}

                def p7s0(t):
                    gt = gpool3.tile([P, CPT, GROW], fp16, tag="gatg")
                    tile_gathers(gt, t3A, t3B, t, GROW, qbase=t)
                    ohT = ohpool.tile([P, CPT * P], f8, tag="ohdT")
                    nc.sync.dma_start(out=ohT[:],
                                      in_=ohdT_d[:, t * CPT * P:(t + 1) * CPT * P])
                    gts[t] = (gt, ohT)

                def p7s1(t):
                    gt, ohT = gts[t]
                    # a_d broadcast to edges: adp_j = ohT_j @ a_d_tile (PE)
                    adps = pspool.tile([P, CPT * HEADS], f32, tag="mm")
                    for j in range(CPT):
                        nc.tensor.matmul(
                            out=adps[:, j * HEADS:(j + 1) * HEADS],
                            lhsT=ohT[:, j * P:(j + 1) * P],
                            rhs=a_d_all[:, t * HEADS:(t + 1) * HEADS],
                            start=True, stop=True)
                    # ex = exp(leaky_relu(a_s + a_d)) batched per tile
                    ea = spool.tile([P, CPT, HEADS], f32, tag="ea")
                    nc.vector.tensor_tensor(
                        out=ea[:], in0=gt[:, :, AS_OFF:AS_OFF + 8].bitcast(f32),
                        in1=adps[:].rearrange("p (a b) -> p a b", b=HEADS), op=OP.add)
                    eaf = ea[:].rearrange("p a b -> p (a b)")
                    nc.vector.scalar_tensor_tensor(
                        out=eaf, in0=eaf, scalar=0.2, in1=eaf,
                        op0=OP.mult, op1=OP.max)
                    ex = spool.tile([P, CPT, HEADS], fp16, tag="ex")
                    nc.scalar.activation(out=ex[:].rearrange("p a b -> p (a b)"),
                                         in_=eaf, func=AF.Exp)
                    exs[t] = ex

                def p7s2a(t):
                    gt, ohT = gts[t]
                    ex = exs.pop(t)
                    # scale rows by ex in place; overwrite a_s slots with ex so
                    # cols [0:516] become [feats*ex | ex] (denominator column)
                    gt4 = gt[:, :, 0:GD].rearrange("p a (h c) -> p a h c", h=HEADS)
                    nc.vector.tensor_tensor(
                        out=gt4, in0=gt4,
                        in1=ex[:].to_broadcast([P, CPT, HEADS, HID]),
                        op=OP.mult)
                    nc.vector.tensor_copy(out=gt[:, :, GD:GD + 4], in_=ex[:])
                    oh = ohpool.tile([P, CPT * P], f8, tag="ohd")
                    nc.sync.dma_start(out=oh[:],
                                      in_=ohd_d[:, t * CPT * P:(t + 1) * CPT * P])
                    ohs[t] = oh

                def p7s2b(t):
                    gt, ohT = gts.pop(t)
                    oh = ohs.pop(t)
                    Ua = psg.tile([P, 256], f32, tag="Ua")
                    Ub = psg.tile([P, 260], f32, tag="Ub")
                    for j in range(CPT):
                        nc.tensor.matmul(out=Ua[:], lhsT=oh[:, j * P:(j + 1) * P],
                                         rhs=gt[:, j, 0:256],
                                         start=(j == 0), stop=(j == CPT - 1))
                        nc.tensor.matmul(out=Ub[:], lhsT=oh[:, j * P:(j + 1) * P],
                                         rhs=gt[:, j, 256:GD + 4],
                                         start=(j == 0), stop=(j == CPT - 1))
                    Uabs[t] = (Ua, Ub)

                def p7s3(t):
                    Ua, Ub = Uabs.pop(t)
                    rden = spool.tile([P, HEADS], f32, tag="rden")
                    nc.vector.reciprocal(out=rden[:], in_=Ub[:, 256:260])
                    gat = wpool.tile([P, GD], fp16, tag="gat")
                    for h in range(HEADS):
                        Uh = Ua if h < 2 else Ub
                        pos = (h % 2) * P
                        nc.scalar.activation(
                            out=gat[:, h * P:(h + 1) * P], in_=Uh[:, pos:pos + P],
                            func=AF.Copy, scale=rden[:, h:h + 1])
                    ao = psg.tile([P, P], f32, tag="Ub")
                    for k in range(4):
                        tp = tppool.tile([P, P], fp16, tag="tp")
                        nc.tensor.transpose(out=tp[:], in_=gat[:, k * P:(k + 1) * P],
                                            identity=ident_t[:])
                        aT = wpool.tile([P, P], fp16, tag="aT")
                        nc.scalar.activation(out=aT[:], in_=tp[:], func=AF.Copy)
                        nc.tensor.matmul(out=ao[:], lhsT=watt_t[:, k, :], rhs=aT[:],
                                         start=(k == 0), stop=(k == 3))
                    aout = wpool.tile([P, P], fp16, tag="aout")
                    nc.scalar.activation(out=aout[:], in_=ao[:],
                                         func=AF.Relu, bias=batt_t[:], scale=1.0)
                    po = pspool.tile([P, OUT_DIM], f32, tag="mm")
                    nc.tensor.matmul(out=po[:], lhsT=aout[:], rhs=wout_t[:],
                                     start=True, stop=True)
                    osb = wpool.tile([P, OUT_DIM], f32, tag="osb")
                    nc.vector.tensor_add(out=osb[:], in0=po[:], in1=bout_t[:])
                    nc.sync.dma_start(out=out_c[t * P:(t + 1) * P, :], in_=osb[:])

                for i in range(TPC + 5):
                    if 5 <= i:
                        p7s3(i - 5)
                    if 4 <= i < TPC + 4:
                        p7s2b(i - 4)
                    if 3 <= i < TPC + 3:
                        p7s2a(i - 3)
                    if 2 <= i < TPC + 2:
                        p7s1(i - 2)
                    if i < TPC:
                        p7s0(i)

    nc.finalize()
    return nc


_CACHE = {}


def kernel(_trace=False, **inputs):
    from concourse import bass_utils
    import ml_dtypes

    fp16 = np.float16

    ei = np.asarray(inputs["edge_index"])
    sidx16, dinv_new, perm, plan, counts, ohd, ohdT = _preprocess(ei)
    CPT = plan["CPT"]

    x = np.asarray(inputs["x"], dtype=np.float32)
    xP = np.zeros((N_PAD, IN_DIM), np.float32)
    xP[perm[:N]] = x
    # host transpose+cast: xT[c] layout [p, h*NPC + n] = x[node n, h*128+p]
    xT = np.ascontiguousarray(
        xP.T.reshape(2, P, NCORES, NPC).transpose(2, 1, 0, 3)
        .reshape(NCORES, P, 2 * NPC)).astype(fp16)

    g = lambda k: np.asarray(inputs[k], dtype=np.float32)
    Wgat = g("Wgat")
    Wg3 = Wgat.reshape(HID, HEADS, HID)
    Vs = np.einsum("khc,hc->kh", Wg3, g("att_src")).astype(np.float32)
    Vd = np.einsum("khc,hc->kh", Wg3, g("att_dst")).astype(np.float32)
    batt_f = (g("bgat") @ g("Watt_out") + g("batt_out")).astype(np.float32)

    bc = lambda v, w: np.tile(np.asarray(v, np.float32)[None, :w], (P, 1))
    Win = g("Win")
    WinT = np.ascontiguousarray(
        Win.reshape(2, P, HID).transpose(1, 0, 2).reshape(P, 2 * HID)).astype(fp16)
    Watt = g("Watt_out")
    WattT = np.ascontiguousarray(
        Watt.reshape(GD // P, P, HID).transpose(1, 0, 2)).astype(fp16)

    common = {
        "ident16": np.eye(P, dtype=fp16),
        "WinT": WinT,
        "bin_pp": g("bin_")[:, None].astype(np.float32),
        "Wg1": g("Wg1").astype(fp16), "Wg2": g("Wg2").astype(fp16),
        "bg1_bc": bc(g("bg1"), HID), "bg2_bc": bc(g("bg2"), HID),
        "g1g_bc": bc(g("g1_gamma"), HID), "g1b_bc": bc(g("g1_beta"), HID),
        "g2g_bc": bc(g("g2_gamma"), HID), "g2b_bc": bc(g("g2_beta"), HID),
        "Wgat": Wgat.astype(fp16),
        "VsVd": np.concatenate([Vs, Vd], axis=1).astype(fp16),
        "WattT": WattT,
        "batt_pp": batt_f[:, None].astype(np.float32),
        "Wout": g("Wout").astype(fp16),
        "bout_bc": bc(g("bout"), OUT_DIM),
        "eps_pp": np.full((P, 1), 1e-5, np.float32),
    }

    key = ("nc", plan["cA"], plan["cB"])
    if key not in _CACHE:
        _CACHE[key] = _build_nc(plan)
    nc = _CACHE[key]

    in_maps = []
    for c in range(NCORES):
        m = dict(common)
        m["xT_c"] = xT[c]
        m["sidx16"] = np.ascontiguousarray(sidx16[c])
        m["counts"] = np.tile(counts[c][None, :], (P, 1)).astype(np.int32)
        m["ohd"] = ohd[c]
        m["ohdT"] = ohdT[c]
        m["dinv_t"] = np.ascontiguousarray(
            dinv_new[c * NPC:(c + 1) * NPC].reshape(TPC, P).T).astype(np.float32)
        in_maps.append(m)

    res = bass_utils.run_bass_kernel_spmd(
        nc, in_maps, core_ids=list(range(NCORES)), trace=_trace)

    outP = np.concatenate([res.results[c]["out_c"] for c in range(NCORES)], axis=0)
    out = outP[perm[:N]]
    if _trace:
        kernel._last_exec_ns = res.exec_time_ns
        kernel._last_res = res
    return out.astype(np.float32)
